# revision 2
# baseline (speedup 1.0000x reference)
"""Distributed Bass kernel V2: pre-LN multi-head attention on 8 TRN2 cores.

Problem: x[2, 2048, 1024] -> LayerNorm -> QKV (16 heads x 64) -> softmax(QK^T/8)V
         -> out proj [1024] + bias.

Sharding (v3, head-sharded): core = (batch b, head-group hg) with 4 heads per
core over ALL 2048 queries. Each core receives the full batch xT plus only its
4 heads' slices of w_qkv / w_out. K/V are projected once per head on exactly
one core (no 4x recompute as in the query-sharded v1). The out projection is
computed as a per-core PARTIAL (contraction over this core's 256 inner
columns); unsharding sums the 4 partials per batch on the host and adds b_out
-- the standard tensor-parallel partial-sum gather.

Per-core schedule:
  Phase A (per 512-token group, pipelined): DMA xT chunk -> x^2 (Pool) ->
  LN stats via ones-vector matmuls on PE -> stat math on DVE (+ACT sqrt).
  The LN affine is FOLDED INTO THE PROJECTIONS: q = a_t*(W^T x)[j,t] +
  c_t*colsum_j + bias_j, realized as one extra accumulation-row matmul with
  stationary [bias; colsum] against moving [ones; c] and a per-token a-scale
  in the PSUM->SBUF copy -- x is never rewritten and the DVE apply pass
  disappears. ln_scale/ln_bias folded into weights/bias rows on the host
  (exact).

  Attention: for each (query-tile qt of 512, head h): score matmuls write
  [128,1024] 2-bank PSUM spans (kc pairs, double buffered), ONE ACT exp per
  span (amortizes the ~172-cycle ACT/PSUM access overhead; exp scale=0.125
  carries the 1/sqrt(64), bias=-2 recenters for fp8-range headroom and
  cancels in the softmax ratio), attn@V accumulates over kc with a ones
  column appended to V giving the denominator for free. Per (qt,h): raw den
  row -> DRAM; unnormalized attn -> SBUF (bf16). Per qt: one [4,512] gather,
  ONE DVE reciprocal, bf16 rcp -> DRAM -> per-head broadcast -> in-place
  bf16 2x-mode multiply. Out-proj partial per qt DMAs straight from PSUM.

  ACT table discipline: phase A uses only Sqrt; attention only Exp (one set
  switch per iteration). ACT Reciprocal is banned (accuracy) -> DVE recip.

PSUM budget: scores 2x2 banks + AV 2 + out-proj/JIT 2 = 8.
"""

import numpy as np
import ml_dtypes

import concourse.bass as bass
import concourse.mybir as mybir
import concourse.tile as tile
from concourse import bacc
from concourse.bass import ts, ds
from concourse.bass_utils import run_bass_kernel_spmd

B, S, D = 2, 2048, 1024
H, DH = 16, 64
INNER = H * DH          # 1024
N_CORES = 8
HPC = 4                 # heads per core
NHC = HPC * DH          # 256 inner cols per core
F32 = mybir.dt.float32
BF16 = mybir.dt.bfloat16
FP8 = mybir.dt.float8e4
AF = mybir.ActivationFunctionType
OP = mybir.AluOpType

NDC = D // 128          # 8 contraction chunks over D
NIC = NHC // 128        # 2 inner-col chunks (its heads)
NKC = S // 128          # 16 kpos chunks
NTG = 4                 # 512-token groups
TG = S // NTG           # 512
NQT = 4                 # query tiles of 512
QT = 512
NTT = QT // 128         # 4 token tiles per query tile... (=4)
EXPB = -2.0             # exp recentering; cancels in softmax ratio


def _build_iter(nc, tc, ext, it):
    (xT_ext, w3_ext, qkcs_ext, vcs_ext, wo_ext, part_ext) = ext

    with tc.tile_pool(name=f"const{it}", bufs=1) as constp, \
         tc.tile_pool(name=f"pers{it}", bufs=1) as pers, \
         tc.tile_pool(name=f"dram{it}", bufs=1, space="DRAM") as dram:

        # ---- weights / constants ----
        w3 = constp.tile([128, NDC, 3 * NHC], BF16)
        nc.sync.dma_start(w3[:], w3_ext[:, :].rearrange("(c p) n -> p c n", p=128))
        wo_sb = constp.tile([128, NIC, D], BF16)
        nc.sync.dma_start(wo_sb[:], wo_ext[:, :].rearrange("(c p) n -> p c n", p=128))
        qkcs = constp.tile([2, 2 * NHC], BF16)   # rows: [bias; colsum] Q,K
        nc.sync.dma_start(qkcs[:], qkcs_ext[:, :])
        vcs = constp.tile([2, NHC], BF16)        # rows: [bias; colsum] V
        nc.sync.dma_start(vcs[:], vcs_ext[:, :])
        onec = constp.tile([2, S], BF16)         # rows: [ones; c per token]
        nc.vector.memset(onec[0:1, :], 1.0)
        a_colT = constp.tile([128, NKC], F32)    # rstd, (kpos%128, kc) layout
        ones_t = constp.tile([128, 1], BF16)
        nc.vector.memset(ones_t[:], 1.0)
        eps_t = constp.tile([128, 1], F32)
        nc.vector.memset(eps_t[:], 1e-6)
        expb_t = constp.tile([128, 1], F32)
        nc.vector.memset(expb_t[:], EXPB)

        # ---- persistent activations ----
        xT = pers.tile([128, NDC, S], BF16)
        QTt = pers.tile([128, NIC, S], BF16)      # Q^T (inner col, q)
        ktf = pers.tile([128, NIC, S], BF16)      # K^T (inner col, kpos)
        vf = pers.tile([128, NKC, HPC * 65], BF16)  # V + ones col per head
        auT = pers.tile([128, NIC, S], BF16)      # attn out (unnorm->norm), transposed

        a_dram = dram.tile([S], F32)
        c_dram = dram.tile([S], BF16)
        den_dram = dram.tile([NQT * HPC, QT], F32)
        rcp_dram = dram.tile([NQT, HPC, QT], F32)

        # ones columns for the in-matmul softmax denominator
        for kc in range(NKC):
            nc.vector.memset(
                vf[:, kc, :].rearrange("p (h c2) -> p h c2", c2=65)[:, :, 64:65],
                1.0)

        # ====== Phase A: LN + Q/K/V projections, pipelined per token group ==
        with tc.tile_pool(name=f"sq{it}", bufs=2) as sqp, \
             tc.tile_pool(name=f"ln{it}", bufs=2) as lnp, \
             tc.tile_pool(name=f"stps{it}", bufs=4, space="PSUM") as stps, \
             tc.tile_pool(name=f"pps{it}", bufs=2, space="PSUM") as pps, \
             tc.tile_pool(name=f"vps{it}", bufs=2, space="PSUM") as vps:
            for tg in range(NTG):
                tsl = ds(tg * TG, TG)
                nc.sync.dma_start(
                    xT[:, :, tsl],
                    xT_ext[:, tsl].rearrange("(c p) t -> p c t", p=128))
                # x^2 on Pool; LN sums via PE with ones stationary
                sq = sqp.tile([128, NDC, TG], BF16, tag="sq")
                nc.gpsimd.tensor_tensor(sq[:], xT[:, :, tsl], xT[:, :, tsl],
                                        op=OP.mult)
                ssum = stps.tile([1, TG], F32, tag="st")
                ssq = stps.tile([1, TG], F32, tag="st")
                for dc in range(NDC):
                    nc.tensor.matmul(ssum[:], ones_t[:], xT[:, dc, tsl],
                                     start=(dc == 0), stop=(dc == NDC - 1))
                for dc in range(NDC):
                    nc.tensor.matmul(ssq[:], ones_t[:], sq[:, dc, :],
                                     start=(dc == 0), stop=(dc == NDC - 1))
                mean = lnp.tile([1, TG], F32, tag="mean")
                nc.vector.tensor_scalar(mean[:], ssum[:], 1.0 / D, None,
                                        op0=OP.mult)
                msq = lnp.tile([1, TG], F32, tag="msq")
                nc.vector.tensor_tensor(msq[:], mean[:], mean[:], op=OP.mult)
                var = lnp.tile([1, TG], F32, tag="var")
                nc.vector.scalar_tensor_tensor(
                    var[:], ssq[:], 1.0 / D, msq[:],
                    op0=OP.mult, op1=OP.subtract)
                std = lnp.tile([1, TG], F32, tag="std")
                nc.scalar.activation(std[:], var[:], AF.Sqrt,
                                     bias=eps_t[0:1, 0:1])
                rstd = lnp.tile([1, TG], F32, tag="rstd")
                with nc.allow_low_precision(reason="bf16 LN affine is ample"):
                    nc.vector.reciprocal(rstd[:], std[:])
                cb = lnp.tile([1, TG], BF16, tag="cb")
                with nc.allow_low_precision(reason="c row feeds bf16 matmul"):
                    nc.vector.scalar_tensor_tensor(
                        cb[:], mean[:], -1.0, rstd[:], op0=OP.mult, op1=OP.mult)
                nc.sync.dma_start(
                    a_dram[tsl].rearrange("(o t) -> o t", o=1), rstd[:])
                nc.sync.dma_start(
                    c_dram[tsl].rearrange("(o t) -> o t", o=1), cb[:])
                # a broadcast row + per-kpos column form; c joins the ones row
                a_bc = lnp.tile([128, TG], F32, tag="a_bc")
                nc.sync.dma_start(
                    a_bc[:],
                    a_dram[tsl].rearrange(
                        "(o t) -> o t", o=1)[0:1, :].to_broadcast((128, TG)))
                nc.sync.dma_start(
                    a_colT[:, ds(tg * (TG // 128), TG // 128)],
                    a_dram[tsl].rearrange("(c p) -> p c", p=128))
                nc.sync.dma_start(
                    onec[1:2, tsl], c_dram[tsl].rearrange("(o t) -> o t", o=1))

                # Q^T and K^T for this token group (2 inner chunks each);
                # the [bias; colsum] x [ones; c] row carries the LN affine
                for qc in range(NIC):
                    for which, dst in ((0, QTt), (1, ktf)):
                        ps = pps.tile([128, TG], F32, tag="proj")
                        nc.tensor.matmul(
                            ps[:], qkcs[0:2, ds(which * NHC + qc * 128, 128)],
                            onec[0:2, tsl], start=True, stop=False)
                        for dc in range(NDC):
                            nc.tensor.matmul(
                                ps[:],
                                w3[:, dc, ds(which * NHC + qc * 128, 128)],
                                xT[:, dc, tsl],
                                start=False, stop=(dc == NDC - 1))
                        nc.vector.tensor_tensor(
                            dst[:, qc, tsl], ps[:], a_bc[:], op=OP.mult)
                # V for the 4 kpos chunks in this group
                for kk in range(TG // 128):
                    kc = tg * (TG // 128) + kk
                    vp = vps.tile([128, NHC], F32, tag="vproj")
                    nc.tensor.matmul(
                        vp[:], onec[0:2, ds(kc * 128, 128)], vcs[0:2, :],
                        start=True, stop=False)
                    for dc in range(NDC):
                        nc.tensor.matmul(
                            vp[:], xT[:, dc, ds(kc * 128, 128)],
                            w3[:, dc, ds(2 * NHC, NHC)],
                            start=False, stop=(dc == NDC - 1))
                    nc.vector.tensor_scalar(
                        vf[:, kc, :].rearrange(
                            "p (h c2) -> p h c2", c2=65)[:, :, 0:64],
                        vp[:].rearrange("p (h d) -> p h d", d=64),
                        a_colT[:, kc:kc + 1], None, op0=OP.mult)

        # ====== Attention + normalize + out-proj, per query tile ======
        # Each qt's normalize/out-proj tail is emitted interleaved into the
        # NEXT qt's score/exp stream so the PE fills ACT's slack instead of
        # stalling it at qt boundaries.
        with tc.tile_pool(name=f"sc{it}", bufs=2, space="PSUM") as scp, \
             tc.tile_pool(name=f"av{it}", bufs=2, space="PSUM") as avp, \
             tc.tile_pool(name=f"po{it}", bufs=2, space="PSUM") as pop, \
             tc.tile_pool(name=f"ex{it}", bufs=3) as exp_, \
             tc.tile_pool(name=f"nrm{it}", bufs=2) as nrmp:
            pending = []

            def emit_recip(qt):
                den_sb = nrmp.tile([HPC, QT], F32, tag="den")
                nc.sync.dma_start(den_sb[:], den_dram[ds(qt * HPC, HPC), :])
                rcp_sb = nrmp.tile([HPC, QT], F32, tag="rcp")
                with nc.allow_low_precision(reason="f32 denom recip"):
                    nc.vector.reciprocal(rcp_sb[:], den_sb[:])
                nc.sync.dma_start(rcp_dram[qt, :, :], rcp_sb[:])

            def emit_norm(qt, h):
                hr = ds((h % 2) * 64, 64)
                hc = h // 2
                rbc = nrmp.tile([128, QT], F32, tag="rbc")
                nc.sync.dma_start(
                    rbc[hr, :],
                    rcp_dram[qt, h:h + 1, :].to_broadcast((64, QT)))
                sl = auT[hr, hc, ds(qt * QT, QT)]
                nc.vector.tensor_tensor(sl, sl, rbc[hr, :], op=OP.mult)

            def emit_oproj(qt, tt):
                tok = ds(qt * QT + tt * 128, 128)
                ostg = nrmp.tile([128, D], F32, tag="ostg")
                for nh in range(2):
                    po = pop.tile([128, 512], F32, tag="po",
                                  name=f"po{qt}_{tt}_{nh}")
                    for ic in range(NIC):
                        nc.tensor.matmul(
                            po[:], auT[:, ic, tok],
                            wo_sb[:, ic, ds(nh * 512, 512)],
                            start=(ic == 0), stop=(ic == NIC - 1))
                    nc.vector.tensor_copy(ostg[:, ds(nh * 512, 512)], po[:])
                nc.sync.dma_start(part_ext[tok, :], ostg[:])

            for qt in range(NQT):
                qsl = ds(qt * QT, QT)
                for h in range(HPC):
                    hr = ds((h % 2) * 64, 64)
                    hc = h // 2
                    av = avp.tile([128, QT], F32, tag="av", name=f"av{qt}_{h}")
                    for kcp in range(NKC // 2):
                        sc = scp.tile([128, 2, QT], F32, tag="sc",
                                      name=f"sc{qt}_{h}_{kcp}")
                        for kcl in range(2):
                            kc = kcp * 2 + kcl
                            nc.tensor.matmul(
                                sc[:, kcl, :],
                                ktf[hr, hc, ds(kc * 128, 128)],
                                QTt[hr, hc, qsl],
                                start=True, stop=True)
                        ex = exp_.tile([128, 2, QT], BF16, tag="ex")
                        nc.scalar.activation(ex[:], sc[:], AF.Exp,
                                             scale=0.125, bias=expb_t[:])
                        for kcl in range(2):
                            kc = kcp * 2 + kcl
                            nc.tensor.matmul(
                                av[0:65, :],
                                vf[:, kc, ds(h * 65, 65)], ex[:, kcl, :],
                                start=(kc == 0), stop=(kc == NKC - 1))
                        if pending and kcp % 2 == 1:
                            pending.pop(0)()
                    # raw denominator row out; unnormalized attn to SBUF
                    dstg = nrmp.tile([65, QT], F32, tag="dstg")
                    nc.vector.tensor_copy(dstg[64:65, :], av[ds(64, 1), :])
                    nc.sync.dma_start(den_dram[ds(qt * HPC + h, 1), :],
                                      dstg[64:65, :])
                    nc.vector.tensor_copy(auT[hr, hc, qsl], av[0:64, :])

                tail = [lambda qt=qt: emit_recip(qt)]
                tail += [lambda qt=qt, h=h2: emit_norm(qt, h)
                         for h2 in range(HPC)]
                tail += [lambda qt=qt, tt=tt2: emit_oproj(qt, tt)
                         for tt2 in range(NTT)]
                pending.extend(tail)
            for fn in pending:
                fn()


def build_bass(n_iters=1):
    nc = bacc.Bacc(None, num_devices=N_CORES)
    xT_ext = nc.declare_dram_parameter("xT", [D, S], BF16, isOutput=False)
    w3_ext = nc.declare_dram_parameter("w3", [D, 3 * NHC], BF16, isOutput=False)
    qkcs_ext = nc.declare_dram_parameter("qkcs", [2, 2 * NHC], BF16,
                                         isOutput=False)
    vcs_ext = nc.declare_dram_parameter("vcs", [2, NHC], BF16, isOutput=False)
    wo_ext = nc.declare_dram_parameter("wo", [NHC, D], BF16, isOutput=False)
    part_ext = nc.declare_dram_parameter("part", [S, D], F32, isOutput=True)
    ext = (xT_ext, w3_ext, qkcs_ext, vcs_ext, wo_ext, part_ext)
    with tile.TileContext(nc) as tc:
        for it in range(n_iters):
            _build_iter(nc, tc, ext, it)
    nc.finalize()
    return nc


def make_in_maps(x, ln_scale, ln_bias, w_qkv, w_out, b_out):
    bf = ml_dtypes.bfloat16
    lns = np.asarray(ln_scale, np.float32)
    lnb = np.asarray(ln_bias, np.float32)
    wq = np.asarray(w_qkv, np.float32) * lns[:, None]   # fold ln scale
    qkvb = (lnb @ np.asarray(w_qkv, np.float32)).astype(np.float32)
    wo_f = np.asarray(w_out, np.float32)
    xTbf = [np.ascontiguousarray(np.asarray(x[b], np.float32).T).astype(bf)
            for b in range(B)]
    in_maps = []
    for core in range(N_CORES):
        b, hg = core // HPC, core % HPC
        csl = slice(hg * NHC, (hg + 1) * NHC)
        w3 = np.concatenate(
            [wq[:, csl], wq[:, INNER:][:, csl], wq[:, 2 * INNER:][:, csl]],
            axis=1).astype(bf)
        b3 = np.concatenate(
            [qkvb[csl], qkvb[INNER:][csl], qkvb[2 * INNER:][csl]])
        cs3 = w3.astype(np.float32).sum(axis=0)   # colsums of the bf16 weights
        qkcs = np.stack([b3[:2 * NHC], cs3[:2 * NHC]]).astype(bf)
        vcs = np.stack([b3[2 * NHC:], cs3[2 * NHC:]]).astype(bf)
        wo = np.ascontiguousarray(wo_f[csl, :]).astype(bf)
        in_maps.append({
            "xT": xTbf[b], "w3": np.ascontiguousarray(w3),
            "qkcs": np.ascontiguousarray(qkcs),
            "vcs": np.ascontiguousarray(vcs), "wo": wo,
        })
    return in_maps


_CACHED_NC = None


def kernel(x, ln_scale, ln_bias, w_qkv, w_out, b_out):
    global _CACHED_NC
    if _CACHED_NC is None:
        _CACHED_NC = build_bass(n_iters=1)
    in_maps = make_in_maps(x, ln_scale, ln_bias, w_qkv, w_out, b_out)
    res = run_bass_kernel_spmd(_CACHED_NC, in_maps, list(range(N_CORES)))
    bo = np.asarray(b_out, np.float32)
    out = np.empty((B, S, D), np.float32)
    for b in range(B):
        acc = res.results[b * HPC]["part"].copy()
        for hg in range(1, HPC):
            acc += res.results[b * HPC + hg]["part"]
        out[b] = acc + bo
    return out
